# revision 60
# baseline (speedup 1.0000x reference)
"""BlanchotianAttention TRN2 kernel: 8 NeuronCores, data-parallel over batch (2)
x tensor-parallel over heads (4 heads/core).

Layout strategy (per core, batch b, head-group hg = heads h0..h0+3):
  - host passes xT = x[b].T  [1024, 2048]
  - stage A-qk: qkvT = w.T @ xT -> QT/KT in [d, seq] layout (head-pair tiles)
    q-weights pre-scaled by dim^-0.5 / temperature_h on host.
  - stage A-v: V = x @ w_v in [seq, d] layout, augmented per head with a ones
    block: V_aug[j, h*128 : h*128+128] = [v_h (64) | ones (64)].
  - stage B/C (per 512-wide i-chunk, per head pair, per 128-wide j-tile):
    S^T = K @ Q^T via row-packed matmuls (2 heads in PE rows 0-63 / 64-127),
    P = exp(S^T) on ACT (one [128,1024] activation covers both heads),
    PV+l fused: matmul(lhsT=[v_h | ones], rhs=P) accumulates attn@v in PSUM
    rows 0-63 and the softmax denominator (broadcast) in rows 64-127.
  - void token: the void QUERY's output row is dropped by the reference, so it
    is never computed. The void KEY/VALUE occupy j=2048 inside j-tile 16,
    zero-padded to 128 rows; a per-partition exp bias of -100 on that tile
    zeroes the pad rows' contributions.
  - normalize: reciprocal of the l rows + cross-base multiply -> O^T pair tile.
  - stage D: y_partial = O_norm @ w_out_shard; host sums partials over the 4
    head-group cores of each batch (+ b_out).

All matmul operands are float32r (full-rate fp32, ~1.2e-4 rounding).

Scheduling notes (cost-model driven):
  - ACT (exp) is the steady-state bottleneck; the jt loop is software-pipelined
    (scores emitted one j-tile ahead) and stage A is interleaved with the first
    i-chunk quarter-by-quarter so ACT starts as early as possible.
  - PSUM slot tags are all explicit per-pair/per-head because the Tile slot
    allocator reuses the most-recently-freed slot (LIFO), which otherwise
    chains consumers onto the newest producer and serializes PE<->ACT.
"""
import sys

sys.path.insert(0, "/opt/trn_rl_repo")

import numpy as np

DIM, HEADS, B, N = 1024, 16, 2, 2048
D = DIM // HEADS          # 64
HPC = HEADS // 4          # heads per core = 4
NJT = 17                  # j tiles (16 full + void/pad tile)
P = 128

_cache = {}


def _build():
    import concourse.bass as bass
    import concourse.mybir as mybir
    import concourse.tile as tile
    from concourse import bacc

    F32 = mybir.dt.float32
    F32R = mybir.dt.float32r
    Exp = mybir.ActivationFunctionType.Exp

    nc = bacc.Bacc("TRN2", target_bir_lowering=False, debug=False)
    xT = nc.dram_tensor("xT", [DIM, N], F32, kind="ExternalInput").ap()
    wqkv = nc.dram_tensor("wqkv", [DIM, 768], F32, kind="ExternalInput").ap()
    wout = nc.dram_tensor("wout", [256, DIM], F32, kind="ExternalInput").ap()
    voidk = nc.dram_tensor("voidk", [2, P], F32, kind="ExternalInput").ap()
    voidv = nc.dram_tensor("voidv", [1, 256], F32, kind="ExternalInput").ap()
    ebias_in = nc.dram_tensor("ebias_in", [P, 1], F32, kind="ExternalInput").ap()
    y = nc.dram_tensor("y", [N, DIM], F32, kind="ExternalOutput").ap()

    KO = DIM // P  # 8 k-tiles

    with tile.TileContext(nc) as tc:
        with tc.tile_pool(name="persist", bufs=1) as pp, \
             tc.tile_pool(name="work", bufs=1) as wp, \
             tc.tile_pool(name="psum", bufs=1, space="PSUM") as ps, \
             tc.tile_pool(name="loadA", bufs=2) as lp:

            # ---- constants ----
            ones = pp.tile([P, D], F32)
            nc.vector.memset(ones[:], 1.0)
            ebias = pp.tile([P, 1], F32)
            nc.sync.dma_start(ebias[:], ebias_in)

            # ---- persistent SBUF tensors ----
            qt = pp.tile([P, 2, N], F32R)              # QT head pairs
            kt = pp.tile([P, 2, NJT * P], F32R)        # KT head pairs (+void+pad)
            va = pp.tile([P, NJT, 512], F32R)          # V_aug per j-tile
            wqkv_r = pp.tile([P, KO, 768], F32R)
            wout_r = pp.tile([P, 2, DIM], F32R)
            xT_r = pp.tile([P, KO, N], F32R)

            # ---- DMA + rounding (order = arrival priority) ----
            # interleave wqk/xT-half0 per ko so A-qk(sc0) accumulation can
            # start as soon as the first k-tile lands
            for ko in range(KO):
                stg = lp.tile([P, 1024], F32, tag="stg")
                nc.gpsimd.dma_start(stg[:, 0:768], wqkv[ko * P:(ko + 1) * P, :])
                nc.vector.tensor_copy(wqkv_r[:, ko, :], stg[:, 0:768])
                stg = lp.tile([P, 1024], F32, tag="stg")
                nc.sync.dma_start(stg[:], xT[ko * P:(ko + 1) * P, 0:1024])
                nc.vector.tensor_copy(xT_r[:, ko, 0:1024], stg[:])
            for ko in range(KO):
                stg = lp.tile([P, 1024], F32, tag="stg")
                nc.sync.dma_start(
                    stg[:], xT[ko * P:(ko + 1) * P, 1024:2048])
                nc.vector.tensor_copy(
                    xT_r[:, ko, 1024:2048], stg[:])

            def emit_late_setup():
                # void k columns + pad zeros; V_aug ones blocks + void row;
                # wout load. Deferred past the sc0 prefix so the DVE stream
                # evacuates the first scores' inputs sooner.
                vkt = lp.tile([P, 2], F32, tag="stg")
                nc.sync.dma_start(vkt[:], voidk.rearrange("a p -> p a"))
                for pair in range(2):
                    nc.vector.tensor_copy(kt[:, pair, 2048:2049],
                                          vkt[:, pair:pair + 1])
                    nc.vector.memset(kt[:, pair, 2049:NJT * P].bitcast(F32), 0.0)
                vvt = lp.tile([1, 256], F32, tag="stg")
                nc.sync.dma_start(vvt[:], voidv)
                va16 = va[:, 16, :]
                nc.vector.memset(va16.bitcast(F32), 0.0)
                nc.vector.tensor_copy(
                    va16.rearrange("p (h c) -> p h c", c=P)[0:1, :, 0:D],
                    vvt[:].rearrange("p (h c) -> p h c", c=D))
                for jt in range(NJT):
                    nc.vector.tensor_copy(
                        va[:, jt, :].rearrange("p (h c) -> p h c", c=P)[:, :, D:P],
                        ones[:, None, :].to_broadcast([P, 4, D]))
                for half in range(2):
                    stg = lp.tile([P, 1024], F32, tag="stg")
                    nc.sync.dma_start(stg[:], wout[half * P:(half + 1) * P, :])
                    nc.vector.tensor_copy(wout_r[:, half, :], stg[:])

            # ---- stage A emit helpers ----
            def emit_aqk_ft(sc, ft):
                acc = ps.tile([P, 1024], F32, tag=f"srot{ft % 2}",
                              name=f"aqk_{sc}_{ft}")
                for ko in range(KO):
                    nc.tensor.matmul(
                        acc[:, 0:512],
                        wqkv_r[:, ko, ft * P:(ft + 1) * P],
                        xT_r[:, ko, sc * 512:(sc + 1) * 512],
                        start=(ko == 0), stop=(ko == KO - 1),
                    )
                if ft < 2:
                    nc.vector.tensor_copy(
                        qt[:, ft, sc * 512:(sc + 1) * 512], acc[:, 0:512])
                else:
                    nc.vector.tensor_copy(
                        kt[:, ft - 2, sc * 512:(sc + 1) * 512], acc[:, 0:512])

            def emit_aqk(sc):
                for ft in range(4):  # 0,1: q pairs; 2,3: k pairs
                    emit_aqk_ft(sc, ft)

            def emit_av(st):
                acc = ps.tile([P, 1024], F32, tag=f"srot{st % 2}",
                              name=f"av_{st}")
                for ko in range(KO):
                    nc.tensor.matmul(
                        acc[:, 0:256],
                        xT_r[:, ko, st * P:(st + 1) * P],
                        wqkv_r[:, ko, 512:768],
                        start=(ko == 0), stop=(ko == KO - 1),
                    )
                nc.vector.tensor_copy(
                    va[:, st, :].rearrange("p (h c) -> p h c", c=P)[:, :, 0:D],
                    acc[:, 0:256].rearrange("p (h c) -> p h c", c=D))

            # ---- stage B/C/D emit helpers ----
            def emit_scores_pair(ic, jt, pair):
                isl = slice(ic * 512, (ic + 1) * 512)
                jsl = slice(jt * P, (jt + 1) * P)
                s_pair = ps.tile([P, 1024], F32, tag=f"srot{pair}",
                                 name=f"s_{ic}_{jt}_{pair}")
                nc.tensor.matmul(
                    s_pair[:, 0:512],
                    kt[0:D, pair, jsl], qt[0:D, pair, isl],
                    start=True, stop=True)
                nc.tensor.matmul(
                    s_pair[:, 512:1024],
                    kt[D:P, pair, jsl], qt[D:P, pair, isl],
                    start=True, stop=True)
                return s_pair

            def emit_scores(ic, jt):
                isl = slice(ic * 512, (ic + 1) * 512)
                jsl = slice(jt * P, (jt + 1) * P)
                tiles = []
                for pair in range(2):
                    s_pair = ps.tile([P, 1024], F32, tag=f"srot{pair}",
                                     name=f"s_{ic}_{jt}_{pair}")
                    nc.tensor.matmul(
                        s_pair[:, 0:512],
                        kt[0:D, pair, jsl], qt[0:D, pair, isl],
                        start=True, stop=True)
                    nc.tensor.matmul(
                        s_pair[:, 512:1024],
                        kt[D:P, pair, jsl], qt[D:P, pair, isl],
                        start=True, stop=True)
                    tiles.append(s_pair)
                return tiles

            def emit_exp_pvl(ic, jt, s_cur, pvl, nxt, mid=None):
                """exp(jt) ; scores(nxt) ; [mid()] ; pvl(jt)."""
                p_tiles = []
                for pair in range(2):
                    p_pair = wp.tile([P, 1024], F32R, tag=f"pexp{pair}",
                                     bufs=3 if pair == 0 else 2,
                                     name=f"p_{ic}_{jt}_{pair}")
                    if jt == 16:
                        nc.scalar.activation(p_pair[:], s_cur[pair][:], Exp,
                                             bias=ebias[:])
                    else:
                        nc.scalar.activation(p_pair[:], s_cur[pair][:], Exp)
                    p_tiles.append(p_pair)
                s_nxt = emit_scores(*nxt) if nxt is not None else None
                if mid is not None:
                    mid()
                for pair in range(2):
                    for hh in range(2):
                        h = 2 * pair + hh
                        nc.tensor.matmul(
                            pvl[h][:],
                            va[:, jt, h * P:(h + 1) * P],
                            p_tiles[pair][:, hh * 512:(hh + 1) * 512],
                            start=(jt == 0), stop=(jt == 16),
                        )
                return s_nxt

            def emit_norm(ic, pvl):
                """normalize + pre-allocate y psum tiles; returns (osb, yps)."""
                osb = [wp.tile([P, 512], F32R, tag=f"osb{pair}",
                               bufs=2, name=f"osb{pair}_{ic}")
                       for pair in range(2)]
                for h in range(4):
                    pair, hh = divmod(h, 2)
                    r_sb = lp.tile([P, 1024], F32, tag="stg",
                                   name=f"rsb_{ic}_{h}")[:, 0:512]
                    nc.vector.reciprocal(r_sb[D:P, :], pvl[h][D:P, :])
                    nc.vector.tensor_tensor(
                        osb[pair][hh * D:(hh + 1) * D, :],
                        pvl[h][0:D, :], r_sb[D:P, :],
                        mybir.AluOpType.mult)
                yps = [ps.tile([P, 512], F32, tag=f"pvl{k % 4}",
                               name=f"y_{ic}_{k}") for k in range(8)]
                return osb, yps

            def emit_outproj(ic, osb, yps, its=range(4), split_q=False):
                for it in its:
                    ysb = wp.tile([P, DIM], F32, tag="ysb", bufs=2,
                                  name=f"ysb_{ic}_{it}")
                    for oc in range(2):
                        yp = yps[it * 2 + oc]
                        for pair in range(2):
                            nc.tensor.matmul(
                                yp[:],
                                osb[pair][:, it * P:(it + 1) * P],
                                wout_r[:, pair, oc * 512:(oc + 1) * 512],
                                start=(pair == 0), stop=(pair == 1),
                            )
                        nc.vector.tensor_copy(ysb[:, oc * 512:(oc + 1) * 512],
                                              yp[:])
                    eng = nc.gpsimd if (split_q and it % 2) else nc.sync
                    eng.dma_start(
                        y[ic * 512 + it * P: ic * 512 + (it + 1) * P, :], ysb[:])

            def alloc_pvl(ic):
                return [ps.tile([P, 512], F32, tag=f"pvl{h}", name=f"pvl{h}_{ic}")
                        for h in range(4)]

            # ---- main schedule ----
            # ic0 interleaved with stage A quarter-by-quarter; ic1..3 plain.
            pvl = alloc_pvl(0)
            emit_aqk_ft(0, 0)
            emit_aqk_ft(0, 2)
            s00_p0 = emit_scores_pair(0, 0, 0)
            emit_aqk_ft(0, 1)
            emit_aqk_ft(0, 3)
            s00_p1 = emit_scores_pair(0, 0, 1)
            emit_late_setup()
            for st in range(0, 4):
                emit_av(st)
            s_cur = [s00_p0, s00_p1]
            for jt in range(0, 3):
                s_cur = emit_exp_pvl(0, jt, s_cur, pvl, (0, jt + 1))
            for q in range(1, 4):
                emit_aqk_ft(q, 2)
                emit_aqk_ft(q, 3)
                for st in range(4 * q, 4 * q + 4):
                    emit_av(st)
                lo, hi = 4 * q - 1, 4 * q + 3   # jts whose next-scores live in sc q
                for jt in range(lo, hi if q < 3 else NJT):
                    nxt = (0, jt + 1) if jt < NJT - 1 else (1, 0)
                    s_cur = emit_exp_pvl(0, jt, s_cur, pvl, nxt)
                if q == 1:
                    # sc1 q-features feed ic1's scores (start at ic0-jt16)
                    emit_aqk_ft(q, 0)
                    emit_aqk_ft(q, 1)

            pvl_prev = pvl
            for ic in range(1, 4):
                osb, yps = emit_norm(ic - 1, pvl_prev)
                pvl = alloc_pvl(ic)
                for jt in range(NJT):
                    if jt == NJT - 1:
                        nxt = (ic + 1, 0) if ic < 3 else None
                    else:
                        nxt = (ic, jt + 1)
                    s_cur = emit_exp_pvl(ic, jt, s_cur, pvl, nxt)
                    if jt == 0:
                        emit_outproj(ic - 1, osb, yps)
                    if jt == 2 and ic < 3:
                        # sc(ic+1) q-features, needed by ic(ic+1)'s scores
                        emit_aqk_ft(ic + 1, 0)
                        emit_aqk_ft(ic + 1, 1)
                pvl_prev = pvl
            osb, yps = emit_norm(3, pvl_prev)
            emit_outproj(3, osb, yps)

    nc.compile()
    return nc


def _prep_inputs(x, w_qkv, w_out, b_out, void_q, void_k, void_v,
                 attention_trace, temperature_factor):
    """Host-side sharding / layout prep. Returns in_maps for 8 cores."""
    temp = np.maximum(1.0 + np.abs(attention_trace) * temperature_factor,
                      1.0).reshape(HEADS).astype(np.float32)
    scale = (DIM ** -0.5) / temp                       # [16] per head
    qcol_scale = np.repeat(scale, D)                   # [1024]
    wq_scaled = (w_qkv[:, 0:DIM] * qcol_scale[None, :]).astype(np.float32)
    wk = w_qkv[:, DIM:2 * DIM]
    wv_full = w_qkv[:, 2 * DIM:3 * DIM]
    vk = void_k.reshape(HEADS, D)
    vv = void_v.reshape(HEADS, D)

    ebias = np.zeros((P, 1), np.float32)
    ebias[1:, 0] = -100.0

    in_maps = []
    for core in range(8):
        b, hg = divmod(core, 4)
        h0 = hg * HPC
        cs = slice(h0 * D, (h0 + HPC) * D)             # 256 feature cols
        in_maps.append({
            "xT": np.ascontiguousarray(x[b].T),
            "wqkv": np.ascontiguousarray(
                np.concatenate([wq_scaled[:, cs], wk[:, cs],
                                wv_full[:, cs]], axis=1)),
            "wout": np.ascontiguousarray(w_out[cs, :]),
            "voidk": np.ascontiguousarray(vk[h0:h0 + HPC].reshape(2, P)),
            "voidv": np.ascontiguousarray(vv[h0:h0 + HPC].reshape(1, 256)),
            "ebias_in": ebias,
        })
    return in_maps


def _run(in_maps, trace=False):
    from concourse import bass_utils
    if "nc" not in _cache:
        _cache["nc"] = _build()
    return bass_utils.run_bass_kernel_spmd(
        _cache["nc"], in_maps, core_ids=list(range(8)), trace=trace)


def kernel(x, w_qkv, w_out, b_out, void_q, void_k, void_v,
           attention_trace, temperature_factor):
    args = [np.asarray(a, dtype=np.float32) for a in
            (x, w_qkv, w_out, b_out, void_q, void_k, void_v,
             attention_trace, temperature_factor)]
    in_maps = _prep_inputs(*args)
    res = _run(in_maps)
    out = np.zeros((B, N, DIM), np.float32)
    for core in range(8):
        b = core // 4
        out[b] += res.results[core]["y"]
    out += args[3][None, None, :]                      # b_out
    return out
